# revision 35
# baseline (speedup 1.0000x reference)
"""GCNNet2 on 8 Trainium2 NeuronCores (Bass/Tile).

Strategy: shard nodes (contiguous 6250-node ranges) across 8 cores; each core
owns the aggregation for its dst range. The normalized adjacency is graph-
static, so the one-hot scatter matrices S (with the dst-side degree factor
folded in) are precomputed on host in bf16 and streamed from DRAM each layer.
m rows are pre-scaled by the src-side degree factor on-chip, so gathered rows
times S gives exactly norm-weighted messages; the b_gcn bias cancels inside
BatchNorm and is dropped. Per layer: m = h @ W (bf16) scaled by dinv, written
to two bounce halves that AllGather separately (half-A aggregation overlaps
the half-B collective); dma_gather calls of 8 edge tiles (1024 rows — the
SWDGE per-call index cap; larger calls hang) fetch
m[src] rows; PE accumulates seed (self-loop diag) + edge one-hot matmuls per
128-dst window in PSUM; BatchNorm via a tiny AllReduce of per-core sums;
fused scale/bias/relu + residual. Global mean pool via indicator matmul +
AllReduce, then the MLP readout (replicated).
"""
import numpy as np

# Problem constants (hardcoded per contract; kernel.py must be self-contained)
N = 50000
E = 800000
DIN = 146
D = 128
G = 64
L = 4
NC = 10
EPS = 1e-5

C = 8          # cores
NL = N // C    # 6250 nodes per core
NT = (NL + 127) // 128                # 49 node tiles (= aggregation windows)
NT_W = [min(128, NL - t * 128) for t in range(NT)]
HLOC = 3200    # local-node split: tiles 0-24 -> half A, 25-48 -> half B
TA = HLOC // 128          # 25 tiles in half A
HA = C * HLOC             # 25600 rows in table A (int16-safe)
HB = C * (NL - HLOC)      # 24400 rows in table B
GRP = 4        # windows per gather group
MAXJ = 8      # max edge tiles per dma_gather call (1024 idx cap; 1280+ hangs)


def _static_structure(counts):
    """counts: [C, NT, 2] per-core edge counts per (window, half).
    Returns core-invariant tile/call structure. Each call carries a
    core-invariant valid-index count V (max over cores): per-core idx
    streams hold [true edges][0-pads to V][-1 to 128j]; the SWDGE ucode
    emits exactly num_idxs_reg=V descriptors, skipping trailing -1 pads."""
    T = np.maximum.reduce(-(-counts // 128), axis=0)  # [NT, 2] ceil/max over cores
    groups = [list(range(g, min(g + GRP, NT))) for g in range(0, NT, GRP)]
    tile_base = {}
    tile_meta = []   # (w, h) per static tile
    gcalls = {}      # (gi, h) -> [(h, t0, j, V)]
    for gi, ws in enumerate(groups):
        for h in (0, 1):
            t0g = len(tile_meta)
            # max-over-core edge count for each bucket, laid out tile-contiguous
            gcnt = np.zeros(0, dtype=np.int64)
            for w in ws:
                tile_base[(w, h)] = len(tile_meta)
                tile_meta.extend([(w, h)] * int(T[w, h]))
                tc_ = np.zeros(int(T[w, h]), dtype=np.int64)
                mx = int(counts[:, w, h].max())
                full = mx // 128
                tc_[:full] = 128
                if full < len(tc_):
                    tc_[full] = mx - full * 128
                gcnt = np.concatenate([gcnt, tc_])
            ntg = len(tile_meta) - t0g
            cl = []
            r = 0
            while r < ntg:
                j = min(MAXJ, ntg - r)
                V = int(gcnt[r:r + j].sum())
                cl.append((h, t0g + r, j, max(V, 1)))
                r += j
            gcalls[(gi, h)] = cl
    TILES = len(tile_meta)
    # Software-pipelined issue order: PREF groups of half-A calls run before
    # the first half-B call, hiding the AG_B collective latency behind A-half
    # gather work. PREF is bounded by the gt/sg pool depth (buffer-reuse WAR
    # vs the in-order PE window chain): pos(B0) <= bufs + calls(g0,A).
    PREF = 2
    order = []
    for gi in range(min(PREF, len(groups))):
        order.extend(gcalls[(gi, 0)])
    for gi in range(len(groups)):
        order.extend(gcalls[(gi, 1)])
        if gi + PREF < len(groups):
            order.extend(gcalls[(gi + PREF, 0)])
    calls = []
    icol = 0
    for (h, t0, j, V) in order:
        calls.append((h, t0, j, icol, V))
        icol += 8 * j
    # map: static tile -> (call index, slot within call)
    tile_call = {}
    for ci, (h, t0, j, off, V) in enumerate(calls):
        for jj in range(j):
            tile_call[t0 + jj] = (ci, jj)
    return dict(T=T, groups=groups, tile_base=tile_base, tile_meta=tile_meta,
                calls=calls, TILES=TILES, IDXCOLS=icol, tile_call=tile_call)


def _preprocess(x, edge_index, batch):
    src = np.asarray(edge_index[0], dtype=np.int64)
    dst = np.asarray(edge_index[1], dtype=np.int64)
    batch = np.asarray(batch, dtype=np.int64)

    deg = (np.bincount(dst, minlength=N) + 1).astype(np.float32)  # + self-loop
    dinv = (1.0 / np.sqrt(deg)).astype(np.float32)

    # gather-table index (two tables split by owner-local offset)
    oc = src // NL
    osl = src % NL
    half = (osl >= HLOC).astype(np.int64)
    idx16 = np.where(half == 0, oc * HLOC + osl,
                     oc * (NL - HLOC) + (osl - HLOC)).astype(np.int16)

    core = dst // NL
    w = (dst % NL) // 128
    dstl = (dst % NL) % 128

    key = (core * NT + w) * 2 + half
    order = np.argsort(key, kind="stable")
    key_s = key[order]
    bounds = np.searchsorted(key_s, np.arange(C * NT * 2 + 1))
    counts = np.zeros((C, NT, 2), dtype=np.int64)
    for c in range(C):
        for ww in range(NT):
            for h in range(2):
                k = (c * NT + ww) * 2 + h
                counts[c, ww, h] = bounds[k + 1] - bounds[k]

    meta = _static_structure(counts)
    T, TILES, IDXCOLS = meta["T"], meta["TILES"], meta["IDXCOLS"]
    tile_base, calls = meta["tile_base"], meta["calls"]

    try:
        import ml_dtypes
        bf16 = ml_dtypes.bfloat16
    except ImportError:  # pragma: no cover
        from jax import numpy as jnp
        bf16 = jnp.bfloat16

    per_core = []
    for c in range(C):
        S = np.zeros((128, TILES * 128), dtype=np.float32)
        flat_idx = np.zeros((TILES, 128), dtype=np.int16)
        for ww in range(NT):
            for h in range(2):
                k = (c * NT + ww) * 2 + h
                el = order[bounds[k]:bounds[k + 1]]
                if len(el) == 0:
                    continue
                tb = tile_base[(ww, h)]
                s = np.arange(len(el))
                ti = tb + s // 128
                slot = s % 128
                S[slot, ti * 128 + dstl[el]] = dinv[dst[el]]
                flat_idx[ti, slot] = idx16[el]
        # pack gather indices per call: idx i of call -> [i%16, off + i//16]
        idx_arr = np.zeros((16, IDXCOLS), dtype=np.int16)
        for (h, t0, j, off, V) in calls:
            seq = flat_idx[t0:t0 + j].reshape(-1)
            idx_arr[:, off:off + 8 * j] = seq.reshape(8 * j, 16).T
        idx_rep = np.tile(idx_arr, (8, 1))

        lo = c * NL
        dinv_l = dinv[lo:lo + NL]
        dinv_p = np.zeros((128, NT), dtype=np.float32)
        sd = np.zeros((128, NT * 128), dtype=np.float32)
        Pm = np.zeros((128, NT * G), dtype=np.float32)
        for t in range(NT):
            cw = NT_W[t]
            dinv_p[:cw, t] = dinv_l[t * 128:t * 128 + cw]
            q = np.arange(cw)
            sd[q, t * 128 + q] = dinv_l[t * 128 + q]
            Pm[q, t * G + batch[lo + t * 128 + q]] = 1.0

        x_c = np.asarray(x[lo:lo + NL], dtype=np.float32).T  # [DIN, NL]
        per_core.append(dict(
            idx=idx_rep,
            S=np.ascontiguousarray(S).astype(bf16),
            sd=sd.astype(bf16),
            Pm=Pm.astype(bf16),
            dinv_p=dinv_p,
            x1_t=np.ascontiguousarray(x_c[:128]).astype(bf16),
            x2_t=np.ascontiguousarray(x_c[128:]).astype(bf16),
        ))

    cnt = np.bincount(batch, minlength=G).astype(np.float32)
    inv_cnt = (1.0 / np.maximum(cnt, 1.0)).astype(np.float32).reshape(G, 1)
    return meta, per_core, inv_cnt, bf16


def _build(meta):
    import concourse.bacc as bacc
    import concourse.mybir as mybir
    import concourse.tile as tile

    f32 = mybir.dt.float32
    bf = mybir.dt.bfloat16
    i16 = mybir.dt.int16
    Alu = mybir.AluOpType
    Act = mybir.ActivationFunctionType
    Axis = mybir.AxisListType

    TILES = meta["TILES"]
    IDXCOLS = meta["IDXCOLS"]
    tile_meta = meta["tile_meta"]
    calls = meta["calls"]
    groups = meta["groups"]
    tile_base = meta["tile_base"]
    T = meta["T"]
    tile_call = meta["tile_call"]

    nc = bacc.Bacc(None, target_bir_lowering=False, num_swdge_queues=4)

    P = {}
    P["x1_t"] = nc.declare_dram_parameter("x1_t", [128, NL], bf, isOutput=False)
    P["x2_t"] = nc.declare_dram_parameter("x2_t", [DIN - 128, NL], bf, isOutput=False)
    P["idx"] = nc.declare_dram_parameter("idx", [128, IDXCOLS], i16, isOutput=False)
    P["S"] = nc.declare_dram_parameter("S", [128, TILES * 128], bf, isOutput=False)
    P["sd"] = nc.declare_dram_parameter("sd", [128, NT * 128], bf, isOutput=False)
    P["Pm"] = nc.declare_dram_parameter("Pm", [128, NT * G], bf, isOutput=False)
    P["dinv_p"] = nc.declare_dram_parameter("dinv_p", [128, NT], f32, isOutput=False)
    P["wemb1"] = nc.declare_dram_parameter("wemb1", [128, D], bf, isOutput=False)
    P["wemb2"] = nc.declare_dram_parameter("wemb2", [DIN - 128, D], bf, isOutput=False)
    P["b_emb"] = nc.declare_dram_parameter("b_emb", [D, 1], f32, isOutput=False)
    P["W_gcn"] = nc.declare_dram_parameter("W_gcn", [L, D, D], bf, isOutput=False)
    P["gamma_t"] = nc.declare_dram_parameter("gamma_t", [D, L], f32, isOutput=False)
    P["beta_t"] = nc.declare_dram_parameter("beta_t", [D, L], f32, isOutput=False)
    P["W_r1"] = nc.declare_dram_parameter("W_r1", [D, D // 2], f32, isOutput=False)
    P["b_r1"] = nc.declare_dram_parameter("b_r1", [D // 2, 1], f32, isOutput=False)
    P["W_r2"] = nc.declare_dram_parameter("W_r2", [D // 2, D // 4], f32, isOutput=False)
    P["b_r2"] = nc.declare_dram_parameter("b_r2", [D // 4, 1], f32, isOutput=False)
    P["W_r3"] = nc.declare_dram_parameter("W_r3", [D // 4, NC], f32, isOutput=False)
    P["b_r3"] = nc.declare_dram_parameter("b_r3", [NC, 1], f32, isOutput=False)
    P["ident"] = nc.declare_dram_parameter("ident", [128, 128], f32, isOutput=False)
    P["identb"] = nc.declare_dram_parameter("identb", [128, 128], bf, isOutput=False)
    P["inv_cnt"] = nc.declare_dram_parameter("inv_cnt", [G, 1], f32, isOutput=False)
    out_p = nc.declare_dram_parameter("out", [NC, G], f32, isOutput=True)

    rg = [list(range(C))]

    with tile.TileContext(nc) as tc:
        with (
            tc.tile_pool(name="const", bufs=1) as cst,
            tc.tile_pool(name="hbuf", bufs=1) as hbuf,
            tc.tile_pool(name="gd", bufs=18) as gd,
            tc.tile_pool(name="sp", bufs=18) as sp,
            tc.tile_pool(name="work", bufs=3) as wk,
            tc.tile_pool(name="xst", bufs=1) as xst,
            tc.tile_pool(name="pag", bufs=4, space="PSUM") as pag,
            tc.tile_pool(name="pmm", bufs=2, space="PSUM") as pmm,
            tc.tile_pool(name="pmo", bufs=1, space="PSUM") as pmo,
            tc.tile_pool(name="dram", bufs=1, space="DRAM") as drp,
        ):
            def load_const(name, shape, dt=f32):
                t = cst.tile(shape, dt, tag=f"c_{name}")
                nc.sync.dma_start(out=t[:], in_=P[name][:])
                return t

            # embedding-critical loads first so h0/m/AG_A start ASAP;
            # aggregation/readout consts follow on the same queue.
            wemb1 = load_const("wemb1", [128, D], bf)
            wemb2 = load_const("wemb2", [DIN - 128, D], bf)
            bemb_sb = load_const("b_emb", [D, 1])
            dinv_sb = load_const("dinv_p", [128, NT])
            wgcn_sb = cst.tile([128, L * D], bf)
            for l in range(L):
                nc.sync.dma_start(out=wgcn_sb[:, l * D:(l + 1) * D],
                                  in_=P["W_gcn"][l])
            x1_sb = xst.tile([128, NL], bf, tag="x1")
            nc.sync.dma_start(out=x1_sb[:], in_=P["x1_t"][:])
            x2_sb = xst.tile([DIN - 128, NL], bf, tag="x2")
            nc.sync.dma_start(out=x2_sb[:], in_=P["x2_t"][:])
            idx_sb = load_const("idx", [128, IDXCOLS], i16)
            sd_sb = load_const("sd", [128, NT * 128], bf)
            pm_sb = load_const("Pm", [128, NT * G], bf)
            gamma_sb = load_const("gamma_t", [D, L])
            beta_sb = load_const("beta_t", [D, L])
            wr1_sb = load_const("W_r1", [D, D // 2])
            br1_sb = load_const("b_r1", [D // 2, 1])
            wr2_sb = load_const("W_r2", [D // 2, D // 4])
            br2_sb = load_const("b_r2", [D // 4, 1])
            wr3_sb = load_const("W_r3", [D // 4, NC])
            br3_sb = load_const("b_r3", [NC, 1])
            ident_sb = load_const("ident", [128, 128])
            identb_sb = load_const("identb", [128, 128], bf)
            invc_sb = load_const("inv_cnt", [G, 1])

            hA = hbuf.tile([128, NT * 128], bf)
            hB = hbuf.tile([128, NT * 128], bf)
            hagg = hbuf.tile([128, NT * 128], f32)
            m_sb = hbuf.tile([128, NT * 128], bf)
            sums = hbuf.tile([128, NT], f32)
            sumsq = hbuf.tile([128, NT], f32)

            m_bounceA = drp.tile([HLOC, D], bf)
            m_bounceB = drp.tile([NL - HLOC, D], bf)
            m_fullA = [drp.tile([HA, D], bf, name=f"m_fullA{l}",
                                addr_space="Shared") for l in range(L)]
            m_fullB = [drp.tile([HB, D], bf, name=f"m_fullB{l}",
                                addr_space="Shared") for l in range(L)]
            stat_in = drp.tile([128, 2], f32)
            stat_out = [drp.tile([128, 2], f32, name=f"stat_out{l}",
                                 addr_space="Shared") for l in range(L)]
            pool_in = drp.tile([G, D], f32)
            pool_out = drp.tile([G, D], f32, addr_space="Shared")

            # ---- embedding: h0_T = W_emb.T @ x_T + b_emb (x resident) ----
            hbufs = [hA, hB]
            qn = [0]

            def m_phase_tile(h_src, l, t):
                """m = dinv * (h @ W_l) for node tile t: PSUM -> bf16 m_sb ->
                bounce-half DMA; triggers the half AllGathers at t=TA-1/NT-1."""
                cw = NT_W[t]
                W_l = wgcn_sb[:, l * D:(l + 1) * D]
                pm = pmm.tile([128, D], f32, tag="pm", name="pm")
                nc.tensor.matmul(out=pm[:cw, :],
                                 lhsT=h_src[:, t * 128:t * 128 + cw],
                                 rhs=W_l, start=True, stop=True)
                nc.vector.tensor_scalar(out=m_sb[:cw, t * D:(t + 1) * D],
                                        in0=pm[:cw, :],
                                        scalar1=dinv_sb[:cw, t:t + 1],
                                        scalar2=None, op0=Alu.mult)
                dma_eng = nc.sync if t % 2 == 0 else nc.scalar
                if t < TA:
                    dma_eng.dma_start(
                        out=m_bounceA[t * 128:t * 128 + cw, :],
                        in_=m_sb[:cw, t * D:(t + 1) * D])
                else:
                    r0 = (t - TA) * 128
                    dma_eng.dma_start(
                        out=m_bounceB[r0:r0 + cw, :],
                        in_=m_sb[:cw, t * D:(t + 1) * D])
                if t == TA - 1:
                    nc.gpsimd.collective_compute(
                        "AllGather", Alu.bypass, replica_groups=rg,
                        ins=[m_bounceA.opt()], outs=[m_fullA[l].opt()])
                if t == NT - 1:
                    nc.gpsimd.collective_compute(
                        "AllGather", Alu.bypass, replica_groups=rg,
                        ins=[m_bounceB.opt()], outs=[m_fullB[l].opt()])

            # embedding + layer-0 m fused per tile
            for t in range(NT):
                c0 = t * 128
                cw = NT_W[t]
                pe = pmm.tile([128, 128], f32, tag="pm", name="pe")
                nc.tensor.matmul(out=pe[:, :cw], lhsT=wemb1[:],
                                 rhs=x1_sb[:, c0:c0 + cw], start=True, stop=False)
                nc.tensor.matmul(out=pe[:, :cw], lhsT=wemb2[:],
                                 rhs=x2_sb[:, c0:c0 + cw], start=False, stop=True)
                nc.scalar.activation(out=hA[:, c0:c0 + cw], in_=pe[:, :cw],
                                     func=Act.Identity, bias=bemb_sb[:, 0:1],
                                     scale=1.0)
                m_phase_tile(hA, 0, t)

            ppool = pmo.tile([G, D], f32, tag="ppool")

            def bn_tail_tile(l, t, h_out):
                """Per-tile tail after BN+residual: next-layer m, or (last
                layer) the pool transpose + indicator accumulation."""
                cw = NT_W[t]
                w0 = t * 128
                if l < L - 1:
                    m_phase_tile(h_out, l + 1, t)
                else:
                    pt = pmm.tile([128, 128], f32, tag="pm", name="pt")
                    nc.tensor.matmul(out=pt[:cw, :],
                                     lhsT=h_out[:, w0:w0 + cw],
                                     rhs=identb_sb[:], start=True, stop=True)
                    hr = wk.tile([128, 128], bf, tag="hr")
                    nc.scalar.activation(out=hr[:cw, :], in_=pt[:cw, :],
                                         func=Act.Identity, bias=0.0, scale=1.0)
                    nc.tensor.matmul(out=ppool[:],
                                     lhsT=pm_sb[:cw, t * G:(t + 1) * G],
                                     rhs=hr[:cw, :],
                                     start=(t == 0), stop=(t == NT - 1))

            # ---- GCN layers ----
            for l in range(L):
                h_in = hbufs[l % 2]
                h_out = hbufs[(l + 1) % 2]

                # issue all gathers + S streams (pipelined via pool bufs)
                gts = {}
                sgs = {}
                for ci, (h, t0, j, off, V) in enumerate(calls):
                    gt = gd.tile([128, MAXJ, D], bf, tag="gt")
                    tab = m_fullA[l] if h == 0 else m_fullB[l]
                    nc.gpsimd.dma_gather(
                        gt[:, :j, :], tab[:], idx_sb[:, off:off + 8 * j],
                        128 * j, 128 * j, D, queue_num=qn[0] % 4)
                    qn[0] += 1
                    sg = sp.tile([128, MAXJ * D], bf, tag="sg")
                    nc.sync.dma_start(out=sg[:, :j * D],
                                      in_=P["S"][:, t0 * D:(t0 + j) * D])
                    for jj in range(j):
                        gts[t0 + jj] = (gt, jj)
                        sgs[t0 + jj] = (sg, jj)

                # aggregate per window: seed opens PSUM, edge tiles accumulate
                for ws in groups:
                    for w in ws:
                        cw = NT_W[w]
                        tiles_w = (list(range(tile_base[(w, 0)],
                                              tile_base[(w, 0)] + int(T[w, 0])))
                                   + list(range(tile_base[(w, 1)],
                                                tile_base[(w, 1)] + int(T[w, 1]))))
                        pw = pag.tile([128, 128], f32, tag="pw")
                        nc.tensor.matmul(
                            out=pw[:], lhsT=m_sb[:cw, w * D:(w + 1) * D],
                            rhs=sd_sb[:cw, w * 128:(w + 1) * 128],
                            start=True, stop=(len(tiles_w) == 0))
                        for i, ti in enumerate(tiles_w):
                            gt, jj = gts[ti]
                            sg, js = sgs[ti]
                            nc.tensor.matmul(
                                out=pw[:], lhsT=gt[:, jj, :],
                                rhs=sg[:, js * D:(js + 1) * D],
                                start=False, stop=(i == len(tiles_w) - 1))
                        w0 = w * 128
                        nc.vector.tensor_copy(out=hagg[:, w0:w0 + cw],
                                              in_=pw[:, :cw])
                        nc.vector.reduce_sum(out=sums[:, w:w + 1],
                                             in_=hagg[:, w0:w0 + cw], axis=Axis.X)
                        sq = wk.tile([128, 128], f32, tag="sq")
                        nc.scalar.square(out=sq[:, :cw], in_=hagg[:, w0:w0 + cw])
                        nc.vector.reduce_sum(out=sumsq[:, w:w + 1],
                                             in_=sq[:, :cw], axis=Axis.X)

                # global BN stats (b_gcn cancels inside BN and is dropped)
                stat_sb = wk.tile([128, 2], f32, tag="stat")
                nc.vector.reduce_sum(out=stat_sb[:, 0:1], in_=sums[:], axis=Axis.X)
                nc.vector.reduce_sum(out=stat_sb[:, 1:2], in_=sumsq[:], axis=Axis.X)
                nc.sync.dma_start(out=stat_in[:], in_=stat_sb[:])
                nc.gpsimd.collective_compute(
                    "AllReduce", Alu.add, replica_groups=rg,
                    ins=[stat_in.opt()], outs=[stat_out[l].opt()])
                stat_g = wk.tile([128, 2], f32, tag="statg")
                nc.sync.dma_start(out=stat_g[:], in_=stat_out[l][:])

                mu = wk.tile([128, 1], f32, tag="mu")
                nc.vector.tensor_scalar(out=mu[:], in0=stat_g[:, 0:1],
                                        scalar1=1.0 / N, scalar2=None, op0=Alu.mult)
                ex2 = wk.tile([128, 1], f32, tag="ex2")
                nc.vector.tensor_scalar(out=ex2[:], in0=stat_g[:, 1:2],
                                        scalar1=1.0 / N, scalar2=None, op0=Alu.mult)
                musq = wk.tile([128, 1], f32, tag="musq")
                nc.vector.tensor_tensor(out=musq[:], in0=mu[:], in1=mu[:], op=Alu.mult)
                var = wk.tile([128, 1], f32, tag="var")
                nc.vector.tensor_tensor(out=var[:], in0=ex2[:], in1=musq[:],
                                        op=Alu.subtract)
                var2 = wk.tile([128, 1], f32, tag="var2")
                nc.vector.tensor_scalar(out=var2[:], in0=var[:], scalar1=float(EPS),
                                        scalar2=None, op0=Alu.add)
                stdv = wk.tile([128, 1], f32, tag="stdv")
                nc.scalar.activation(out=stdv[:], in_=var2[:], func=Act.Sqrt)
                rinv = wk.tile([128, 1], f32, tag="rinv")
                nc.vector.reciprocal(out=rinv[:], in_=stdv[:])
                a_t = wk.tile([128, 1], f32, tag="a_t")
                nc.vector.tensor_tensor(out=a_t[:], in0=gamma_sb[:, l:l + 1],
                                        in1=rinv[:], op=Alu.mult)
                t1 = wk.tile([128, 1], f32, tag="t1")
                nc.vector.tensor_tensor(out=t1[:], in0=mu[:], in1=a_t[:], op=Alu.mult)
                b2 = wk.tile([128, 1], f32, tag="b2")
                nc.vector.tensor_tensor(out=b2[:], in0=beta_sb[:, l:l + 1],
                                        in1=t1[:], op=Alu.subtract)

                # fused BN apply + residual (512-wide slabs to amortize
                # per-instruction overhead) + next-layer m (or final pool)
                for t0c in range(0, NT, 4):
                    tc_n = min(4, NT - t0c)
                    c0 = t0c * 128
                    cwc = sum(NT_W[t0c + i] for i in range(tc_n))
                    rl = wk.tile([128, 512], bf, tag="rl")
                    nc.scalar.activation(out=rl[:, :cwc], in_=hagg[:, c0:c0 + cwc],
                                         func=Act.Relu, bias=b2[:, 0:1],
                                         scale=a_t[:, 0:1])
                    nc.vector.tensor_add(out=h_out[:, c0:c0 + cwc],
                                         in0=rl[:, :cwc], in1=h_in[:, c0:c0 + cwc])
                    for t in range(t0c, t0c + tc_n):
                        bn_tail_tile(l, t, h_out)

            # ---- MLP readout ----
            pool_sb = wk.tile([G, D], f32, tag="pool_sb")
            nc.vector.tensor_copy(out=pool_sb[:], in_=ppool[:])
            nc.sync.dma_start(out=pool_in[:], in_=pool_sb[:])
            nc.gpsimd.collective_compute(
                "AllReduce", Alu.add, replica_groups=rg,
                ins=[pool_in.opt()], outs=[pool_out.opt()])
            pg = wk.tile([G, D], f32, tag="pg")
            nc.sync.dma_start(out=pg[:], in_=pool_out[:])
            hg = wk.tile([G, D], f32, tag="hg")
            nc.vector.tensor_scalar(out=hg[:], in0=pg[:], scalar1=invc_sb[:, 0:1],
                                    scalar2=None, op0=Alu.mult)
            pt2 = pmo.tile([128, G], f32, tag="pmlp")
            nc.tensor.matmul(out=pt2[:], lhsT=hg[:], rhs=ident_sb[:G, :G],
                             start=True, stop=True)
            hgT = wk.tile([128, G], f32, tag="hgT")
            nc.vector.tensor_copy(out=hgT[:], in_=pt2[:])
            pz1 = pmo.tile([D // 2, G], f32, tag="pmlp")
            nc.tensor.matmul(out=pz1[:], lhsT=wr1_sb[:], rhs=hgT[:],
                             start=True, stop=True)
            z1 = wk.tile([D // 2, G], f32, tag="z1")
            nc.scalar.activation(out=z1[:], in_=pz1[:], func=Act.Relu,
                                 bias=br1_sb[:, 0:1], scale=1.0)
            pz2 = pmo.tile([D // 4, G], f32, tag="pmlp")
            nc.tensor.matmul(out=pz2[:], lhsT=wr2_sb[:], rhs=z1[:],
                             start=True, stop=True)
            z2 = wk.tile([D // 4, G], f32, tag="z2")
            nc.scalar.activation(out=z2[:], in_=pz2[:], func=Act.Relu,
                                 bias=br2_sb[:, 0:1], scale=1.0)
            pz3 = pmo.tile([NC, G], f32, tag="pmlp")
            nc.tensor.matmul(out=pz3[:], lhsT=wr3_sb[:], rhs=z2[:],
                             start=True, stop=True)
            z3 = wk.tile([NC, G], f32, tag="z3")
            nc.scalar.activation(out=z3[:], in_=pz3[:], func=Act.Identity,
                                 bias=br3_sb[:, 0:1], scale=1.0)
            nc.sync.dma_start(out=out_p[:], in_=z3[:])

    nc.compile()
    return nc


_CACHE = {}


def kernel(x, edge_index, batch, W_emb, b_emb, W_gcn, b_gcn,
           bn_gamma, bn_beta, W_r1, b_r1, W_r2, b_r2, W_r3, b_r3):
    import os
    from concourse.bass_utils import run_bass_kernel_spmd
    trace = bool(os.environ.get("BASS_KERNEL_TRACE"))

    x = np.asarray(x, dtype=np.float32)
    meta, per_core, inv_cnt, bf16 = _preprocess(x, edge_index, batch)

    key = (meta["TILES"], meta["IDXCOLS"], tuple(map(tuple, meta["T"])))
    if key not in _CACHE:
        _CACHE[key] = _build(meta)
    nc = _CACHE[key]

    W_emb = np.asarray(W_emb, np.float32)
    shared = dict(
        wemb1=np.ascontiguousarray(W_emb[:128]).astype(bf16),
        wemb2=np.ascontiguousarray(W_emb[128:]).astype(bf16),
        b_emb=np.asarray(b_emb, np.float32).reshape(D, 1),
        W_gcn=np.asarray(W_gcn, np.float32).astype(bf16),
        gamma_t=np.ascontiguousarray(np.asarray(bn_gamma, np.float32).T),
        beta_t=np.ascontiguousarray(np.asarray(bn_beta, np.float32).T),
        W_r1=np.asarray(W_r1, np.float32),
        b_r1=np.asarray(b_r1, np.float32).reshape(D // 2, 1),
        W_r2=np.asarray(W_r2, np.float32),
        b_r2=np.asarray(b_r2, np.float32).reshape(D // 4, 1),
        W_r3=np.asarray(W_r3, np.float32),
        b_r3=np.asarray(b_r3, np.float32).reshape(NC, 1),
        ident=np.eye(128, dtype=np.float32),
        identb=np.eye(128, dtype=np.float32).astype(bf16),
        inv_cnt=inv_cnt,
    )
    in_maps = []
    for c in range(C):
        m = dict(shared)
        m.update(per_core[c])
        in_maps.append(m)

    res = run_bass_kernel_spmd(nc, in_maps, core_ids=list(range(C)), trace=trace)
    if trace and res.exec_time_ns is not None:
        print(f"HW exec time: {res.exec_time_ns} ns")
    out = res.results[0]["out"]  # [NC, G]
    return np.ascontiguousarray(out.T.astype(np.float32))  # [G, NC]


# revision 37
# speedup vs baseline: 1.0501x; 1.0501x over previous
"""GCNNet2 on 8 Trainium2 NeuronCores (Bass/Tile).

Strategy: shard nodes (contiguous 6250-node ranges) across 8 cores; each core
owns the aggregation for its dst range. The normalized adjacency is graph-
static, so the one-hot scatter matrices S (with the dst-side degree factor
folded in) are precomputed on host in bf16 and streamed from DRAM each layer.
m rows are pre-scaled by the src-side degree factor on-chip, so gathered rows
times S gives exactly norm-weighted messages; the b_gcn bias cancels inside
BatchNorm and is dropped. Per layer: m = h @ W (bf16) scaled by dinv, written
to two bounce halves that AllGather separately (half-A aggregation overlaps
the half-B collective); dma_gather calls of 8 edge tiles (1024 rows — the
SWDGE per-call index cap; larger calls hang) fetch
m[src] rows; PE accumulates seed (self-loop diag) + edge one-hot matmuls per
128-dst window in PSUM; BatchNorm via a tiny AllReduce of per-core sums;
fused scale/bias/relu + residual. Global mean pool via indicator matmul +
AllReduce, then the MLP readout (replicated).
"""
import numpy as np

# Problem constants (hardcoded per contract; kernel.py must be self-contained)
N = 50000
E = 800000
DIN = 146
D = 128
G = 64
L = 4
NC = 10
EPS = 1e-5

C = 8          # cores
NL = N // C    # 6250 nodes per core
NT = (NL + 127) // 128                # 49 node tiles (= aggregation windows)
NT_W = [min(128, NL - t * 128) for t in range(NT)]
HLOC = 3200    # local-node split: tiles 0-24 -> half A, 25-48 -> half B
TA = HLOC // 128          # 25 tiles in half A
HA = C * HLOC             # 25600 rows in table A (int16-safe)
HB = C * (NL - HLOC)      # 24400 rows in table B
GRP = 4        # windows per gather group
MAXJ = 8      # max edge tiles per dma_gather call (1024 idx cap; 1280+ hangs)


def _static_structure(counts):
    """counts: [C, NT, 2] per-core edge counts per (window, half).
    Returns core-invariant tile/call structure. Each call carries a
    core-invariant valid-index count V (max over cores): per-core idx
    streams hold [true edges][0-pads to V][-1 to 128j]; the SWDGE ucode
    emits exactly num_idxs_reg=V descriptors, skipping trailing -1 pads."""
    T = np.maximum.reduce(-(-counts // 128), axis=0)  # [NT, 2] ceil/max over cores
    groups = [list(range(g, min(g + GRP, NT))) for g in range(0, NT, GRP)]
    tile_base = {}
    tile_meta = []   # (w, h) per static tile
    gcalls = {}      # (gi, h) -> [(h, t0, j, V)]
    for gi, ws in enumerate(groups):
        for h in (0, 1):
            t0g = len(tile_meta)
            # max-over-core edge count for each bucket, laid out tile-contiguous
            gcnt = np.zeros(0, dtype=np.int64)
            for w in ws:
                tile_base[(w, h)] = len(tile_meta)
                tile_meta.extend([(w, h)] * int(T[w, h]))
                tc_ = np.zeros(int(T[w, h]), dtype=np.int64)
                mx = int(counts[:, w, h].max())
                full = mx // 128
                tc_[:full] = 128
                if full < len(tc_):
                    tc_[full] = mx - full * 128
                gcnt = np.concatenate([gcnt, tc_])
            ntg = len(tile_meta) - t0g
            cl = []
            r = 0
            while r < ntg:
                j = min(MAXJ, ntg - r)
                V = int(gcnt[r:r + j].sum())
                cl.append((h, t0g + r, j, max(V, 1)))
                r += j
            gcalls[(gi, h)] = cl
    TILES = len(tile_meta)
    # Software-pipelined issue order: PREF groups of half-A calls run before
    # the first half-B call, hiding the AG_B collective latency behind A-half
    # gather work. PREF is bounded by the gt/sg pool depth (buffer-reuse WAR
    # vs the in-order PE window chain): pos(B0) <= bufs + calls(g0,A).
    PREF = 1
    order = []
    for gi in range(min(PREF, len(groups))):
        order.extend(gcalls[(gi, 0)])
    for gi in range(len(groups)):
        order.extend(gcalls[(gi, 1)])
        if gi + PREF < len(groups):
            order.extend(gcalls[(gi + PREF, 0)])
    calls = []
    icol = 0
    for (h, t0, j, V) in order:
        calls.append((h, t0, j, icol, V))
        icol += 8 * j
    # map: static tile -> (call index, slot within call)
    tile_call = {}
    for ci, (h, t0, j, off, V) in enumerate(calls):
        for jj in range(j):
            tile_call[t0 + jj] = (ci, jj)
    return dict(T=T, groups=groups, tile_base=tile_base, tile_meta=tile_meta,
                calls=calls, TILES=TILES, IDXCOLS=icol, tile_call=tile_call)


def _preprocess(x, edge_index, batch):
    src = np.asarray(edge_index[0], dtype=np.int64)
    dst = np.asarray(edge_index[1], dtype=np.int64)
    batch = np.asarray(batch, dtype=np.int64)

    deg = (np.bincount(dst, minlength=N) + 1).astype(np.float32)  # + self-loop
    dinv = (1.0 / np.sqrt(deg)).astype(np.float32)

    # gather-table index (two tables split by owner-local offset)
    oc = src // NL
    osl = src % NL
    half = (osl >= HLOC).astype(np.int64)
    idx16 = np.where(half == 0, oc * HLOC + osl,
                     oc * (NL - HLOC) + (osl - HLOC)).astype(np.int16)

    core = dst // NL
    w = (dst % NL) // 128
    dstl = (dst % NL) % 128

    key = (core * NT + w) * 2 + half
    order = np.argsort(key, kind="stable")
    key_s = key[order]
    bounds = np.searchsorted(key_s, np.arange(C * NT * 2 + 1))
    counts = np.zeros((C, NT, 2), dtype=np.int64)
    for c in range(C):
        for ww in range(NT):
            for h in range(2):
                k = (c * NT + ww) * 2 + h
                counts[c, ww, h] = bounds[k + 1] - bounds[k]

    meta = _static_structure(counts)
    T, TILES, IDXCOLS = meta["T"], meta["TILES"], meta["IDXCOLS"]
    tile_base, calls = meta["tile_base"], meta["calls"]

    try:
        import ml_dtypes
        bf16 = ml_dtypes.bfloat16
    except ImportError:  # pragma: no cover
        from jax import numpy as jnp
        bf16 = jnp.bfloat16

    per_core = []
    for c in range(C):
        S = np.zeros((128, TILES * 128), dtype=np.float32)
        flat_idx = np.zeros((TILES, 128), dtype=np.int16)
        for ww in range(NT):
            for h in range(2):
                k = (c * NT + ww) * 2 + h
                el = order[bounds[k]:bounds[k + 1]]
                if len(el) == 0:
                    continue
                tb = tile_base[(ww, h)]
                s = np.arange(len(el))
                ti = tb + s // 128
                slot = s % 128
                S[slot, ti * 128 + dstl[el]] = dinv[dst[el]]
                flat_idx[ti, slot] = idx16[el]
        # pack gather indices per call: idx i of call -> [i%16, off + i//16]
        idx_arr = np.zeros((16, IDXCOLS), dtype=np.int16)
        for (h, t0, j, off, V) in calls:
            seq = flat_idx[t0:t0 + j].reshape(-1)
            idx_arr[:, off:off + 8 * j] = seq.reshape(8 * j, 16).T
        idx_rep = np.tile(idx_arr, (8, 1))

        lo = c * NL
        dinv_l = dinv[lo:lo + NL]
        dinv_p = np.zeros((128, NT), dtype=np.float32)
        sd = np.zeros((128, NT * 128), dtype=np.float32)
        Pm = np.zeros((128, NT * G), dtype=np.float32)
        for t in range(NT):
            cw = NT_W[t]
            dinv_p[:cw, t] = dinv_l[t * 128:t * 128 + cw]
            q = np.arange(cw)
            sd[q, t * 128 + q] = dinv_l[t * 128 + q]
            Pm[q, t * G + batch[lo + t * 128 + q]] = 1.0

        x_c = np.asarray(x[lo:lo + NL], dtype=np.float32).T  # [DIN, NL]
        per_core.append(dict(
            idx=idx_rep,
            S=np.ascontiguousarray(S).astype(bf16),
            sd=sd.astype(bf16),
            Pm=Pm.astype(bf16),
            dinv_p=dinv_p,
            x1_t=np.ascontiguousarray(x_c[:128]).astype(bf16),
            x2_t=np.ascontiguousarray(x_c[128:]).astype(bf16),
        ))

    cnt = np.bincount(batch, minlength=G).astype(np.float32)
    inv_cnt = (1.0 / np.maximum(cnt, 1.0)).astype(np.float32).reshape(G, 1)
    return meta, per_core, inv_cnt, bf16


def _build(meta):
    import concourse.bacc as bacc
    import concourse.mybir as mybir
    import concourse.tile as tile

    f32 = mybir.dt.float32
    bf = mybir.dt.bfloat16
    i16 = mybir.dt.int16
    Alu = mybir.AluOpType
    Act = mybir.ActivationFunctionType
    Axis = mybir.AxisListType

    TILES = meta["TILES"]
    IDXCOLS = meta["IDXCOLS"]
    tile_meta = meta["tile_meta"]
    calls = meta["calls"]
    groups = meta["groups"]
    tile_base = meta["tile_base"]
    T = meta["T"]
    tile_call = meta["tile_call"]

    nc = bacc.Bacc(None, target_bir_lowering=False, num_swdge_queues=4)

    P = {}
    P["x1_t"] = nc.declare_dram_parameter("x1_t", [128, NL], bf, isOutput=False)
    P["x2_t"] = nc.declare_dram_parameter("x2_t", [DIN - 128, NL], bf, isOutput=False)
    P["idx"] = nc.declare_dram_parameter("idx", [128, IDXCOLS], i16, isOutput=False)
    P["S"] = nc.declare_dram_parameter("S", [128, TILES * 128], bf, isOutput=False)
    P["sd"] = nc.declare_dram_parameter("sd", [128, NT * 128], bf, isOutput=False)
    P["Pm"] = nc.declare_dram_parameter("Pm", [128, NT * G], bf, isOutput=False)
    P["dinv_p"] = nc.declare_dram_parameter("dinv_p", [128, NT], f32, isOutput=False)
    P["wemb1"] = nc.declare_dram_parameter("wemb1", [128, D], bf, isOutput=False)
    P["wemb2"] = nc.declare_dram_parameter("wemb2", [DIN - 128, D], bf, isOutput=False)
    P["b_emb"] = nc.declare_dram_parameter("b_emb", [D, 1], f32, isOutput=False)
    P["W_gcn"] = nc.declare_dram_parameter("W_gcn", [L, D, D], bf, isOutput=False)
    P["gamma_t"] = nc.declare_dram_parameter("gamma_t", [D, L], f32, isOutput=False)
    P["beta_t"] = nc.declare_dram_parameter("beta_t", [D, L], f32, isOutput=False)
    P["W_r1"] = nc.declare_dram_parameter("W_r1", [D, D // 2], f32, isOutput=False)
    P["b_r1"] = nc.declare_dram_parameter("b_r1", [D // 2, 1], f32, isOutput=False)
    P["W_r2"] = nc.declare_dram_parameter("W_r2", [D // 2, D // 4], f32, isOutput=False)
    P["b_r2"] = nc.declare_dram_parameter("b_r2", [D // 4, 1], f32, isOutput=False)
    P["W_r3"] = nc.declare_dram_parameter("W_r3", [D // 4, NC], f32, isOutput=False)
    P["b_r3"] = nc.declare_dram_parameter("b_r3", [NC, 1], f32, isOutput=False)
    P["ident"] = nc.declare_dram_parameter("ident", [128, 128], f32, isOutput=False)
    P["identb"] = nc.declare_dram_parameter("identb", [128, 128], bf, isOutput=False)
    P["inv_cnt"] = nc.declare_dram_parameter("inv_cnt", [G, 1], f32, isOutput=False)
    out_p = nc.declare_dram_parameter("out", [NC, G], f32, isOutput=True)

    rg = [list(range(C))]

    with tile.TileContext(nc) as tc:
        with (
            tc.tile_pool(name="const", bufs=1) as cst,
            tc.tile_pool(name="hbuf", bufs=1) as hbuf,
            tc.tile_pool(name="gd", bufs=12) as gd,
            tc.tile_pool(name="sp", bufs=12) as sp,
            tc.tile_pool(name="work", bufs=3) as wk,
            tc.tile_pool(name="xst", bufs=1) as xst,
            tc.tile_pool(name="pag", bufs=4, space="PSUM") as pag,
            tc.tile_pool(name="pmm", bufs=2, space="PSUM") as pmm,
            tc.tile_pool(name="pmo", bufs=1, space="PSUM") as pmo,
            tc.tile_pool(name="dram", bufs=1, space="DRAM") as drp,
        ):
            def load_const(name, shape, dt=f32):
                t = cst.tile(shape, dt, tag=f"c_{name}")
                nc.sync.dma_start(out=t[:], in_=P[name][:])
                return t

            # embedding-critical loads first so h0/m/AG_A start ASAP;
            # aggregation/readout consts follow on the same queue.
            wemb1 = load_const("wemb1", [128, D], bf)
            wemb2 = load_const("wemb2", [DIN - 128, D], bf)
            bemb_sb = load_const("b_emb", [D, 1])
            dinv_sb = load_const("dinv_p", [128, NT])
            wgcn_sb = cst.tile([128, L * D], bf)
            for l in range(L):
                nc.sync.dma_start(out=wgcn_sb[:, l * D:(l + 1) * D],
                                  in_=P["W_gcn"][l])
            x1_sb = xst.tile([128, NL], bf, tag="x1")
            nc.sync.dma_start(out=x1_sb[:], in_=P["x1_t"][:])
            x2_sb = xst.tile([DIN - 128, NL], bf, tag="x2")
            nc.sync.dma_start(out=x2_sb[:], in_=P["x2_t"][:])
            idx_sb = load_const("idx", [128, IDXCOLS], i16)
            sd_sb = load_const("sd", [128, NT * 128], bf)
            pm_sb = load_const("Pm", [128, NT * G], bf)
            gamma_sb = load_const("gamma_t", [D, L])
            beta_sb = load_const("beta_t", [D, L])
            wr1_sb = load_const("W_r1", [D, D // 2])
            br1_sb = load_const("b_r1", [D // 2, 1])
            wr2_sb = load_const("W_r2", [D // 2, D // 4])
            br2_sb = load_const("b_r2", [D // 4, 1])
            wr3_sb = load_const("W_r3", [D // 4, NC])
            br3_sb = load_const("b_r3", [NC, 1])
            ident_sb = load_const("ident", [128, 128])
            identb_sb = load_const("identb", [128, 128], bf)
            invc_sb = load_const("inv_cnt", [G, 1])

            hA = hbuf.tile([128, NT * 128], bf)
            hB = hbuf.tile([128, NT * 128], bf)
            hagg = hbuf.tile([128, NT * 128], f32)
            m_sb = hbuf.tile([128, NT * 128], bf)
            sums = hbuf.tile([128, NT], f32)
            sumsq = hbuf.tile([128, NT], f32)

            m_bounceA = drp.tile([HLOC, D], bf)
            m_bounceB = drp.tile([NL - HLOC, D], bf)
            m_fullA = [drp.tile([HA, D], bf, name=f"m_fullA{l}",
                                addr_space="Shared") for l in range(L)]
            m_fullB = [drp.tile([HB, D], bf, name=f"m_fullB{l}",
                                addr_space="Shared") for l in range(L)]
            stat_in = drp.tile([128, 2], f32)
            stat_out = [drp.tile([128, 2], f32, name=f"stat_out{l}",
                                 addr_space="Shared") for l in range(L)]
            pool_in = drp.tile([G, D], f32)
            pool_out = drp.tile([G, D], f32, addr_space="Shared")

            # ---- embedding: h0_T = W_emb.T @ x_T + b_emb (x resident) ----
            hbufs = [hA, hB]
            qn = [0]

            def m_phase_tile(h_src, l, t):
                """m = dinv * (h @ W_l) for node tile t: PSUM -> bf16 m_sb ->
                bounce-half DMA; triggers the half AllGathers at t=TA-1/NT-1."""
                cw = NT_W[t]
                W_l = wgcn_sb[:, l * D:(l + 1) * D]
                pm = pmm.tile([128, D], f32, tag="pm", name="pm")
                nc.tensor.matmul(out=pm[:cw, :],
                                 lhsT=h_src[:, t * 128:t * 128 + cw],
                                 rhs=W_l, start=True, stop=True)
                nc.vector.tensor_scalar(out=m_sb[:cw, t * D:(t + 1) * D],
                                        in0=pm[:cw, :],
                                        scalar1=dinv_sb[:cw, t:t + 1],
                                        scalar2=None, op0=Alu.mult)
                dma_eng = nc.sync if t % 2 == 0 else nc.scalar
                if t < TA:
                    dma_eng.dma_start(
                        out=m_bounceA[t * 128:t * 128 + cw, :],
                        in_=m_sb[:cw, t * D:(t + 1) * D])
                else:
                    r0 = (t - TA) * 128
                    dma_eng.dma_start(
                        out=m_bounceB[r0:r0 + cw, :],
                        in_=m_sb[:cw, t * D:(t + 1) * D])
                if t == TA - 1:
                    nc.gpsimd.collective_compute(
                        "AllGather", Alu.bypass, replica_groups=rg,
                        ins=[m_bounceA.opt()], outs=[m_fullA[l].opt()])
                if t == NT - 1:
                    nc.gpsimd.collective_compute(
                        "AllGather", Alu.bypass, replica_groups=rg,
                        ins=[m_bounceB.opt()], outs=[m_fullB[l].opt()])

            # embedding + layer-0 m fused per tile
            for t in range(NT):
                c0 = t * 128
                cw = NT_W[t]
                pe = pmm.tile([128, 128], f32, tag="pm", name="pe")
                nc.tensor.matmul(out=pe[:, :cw], lhsT=wemb1[:],
                                 rhs=x1_sb[:, c0:c0 + cw], start=True, stop=False)
                nc.tensor.matmul(out=pe[:, :cw], lhsT=wemb2[:],
                                 rhs=x2_sb[:, c0:c0 + cw], start=False, stop=True)
                nc.scalar.activation(out=hA[:, c0:c0 + cw], in_=pe[:, :cw],
                                     func=Act.Identity, bias=bemb_sb[:, 0:1],
                                     scale=1.0)
                m_phase_tile(hA, 0, t)

            ppool = pmo.tile([G, D], f32, tag="ppool")

            def bn_tail_tile(l, t, h_out):
                """Per-tile tail after BN+residual: next-layer m, or (last
                layer) the pool transpose + indicator accumulation."""
                cw = NT_W[t]
                w0 = t * 128
                if l < L - 1:
                    m_phase_tile(h_out, l + 1, t)
                else:
                    pt = pmm.tile([128, 128], f32, tag="pm", name="pt")
                    nc.tensor.matmul(out=pt[:cw, :],
                                     lhsT=h_out[:, w0:w0 + cw],
                                     rhs=identb_sb[:], start=True, stop=True)
                    hr = wk.tile([128, 128], bf, tag="hr")
                    nc.scalar.activation(out=hr[:cw, :], in_=pt[:cw, :],
                                         func=Act.Identity, bias=0.0, scale=1.0)
                    nc.tensor.matmul(out=ppool[:],
                                     lhsT=pm_sb[:cw, t * G:(t + 1) * G],
                                     rhs=hr[:cw, :],
                                     start=(t == 0), stop=(t == NT - 1))

            # ---- GCN layers ----
            for l in range(L):
                h_in = hbufs[l % 2]
                h_out = hbufs[(l + 1) % 2]

                # issue all gathers + S streams (pipelined via pool bufs)
                gts = {}
                sgs = {}
                for ci, (h, t0, j, off, V) in enumerate(calls):
                    gt = gd.tile([128, MAXJ, D], bf, tag="gt")
                    tab = m_fullA[l] if h == 0 else m_fullB[l]
                    nc.gpsimd.dma_gather(
                        gt[:, :j, :], tab[:], idx_sb[:, off:off + 8 * j],
                        128 * j, 128 * j, D, queue_num=qn[0] % 4)
                    qn[0] += 1
                    sg = sp.tile([128, MAXJ * D], bf, tag="sg")
                    nc.sync.dma_start(out=sg[:, :j * D],
                                      in_=P["S"][:, t0 * D:(t0 + j) * D])
                    for jj in range(j):
                        gts[t0 + jj] = (gt, jj)
                        sgs[t0 + jj] = (sg, jj)

                # aggregate per window: seed opens PSUM, edge tiles accumulate
                for ws in groups:
                    for w in ws:
                        cw = NT_W[w]
                        tiles_w = (list(range(tile_base[(w, 0)],
                                              tile_base[(w, 0)] + int(T[w, 0])))
                                   + list(range(tile_base[(w, 1)],
                                                tile_base[(w, 1)] + int(T[w, 1]))))
                        pw = pag.tile([128, 128], f32, tag="pw")
                        nc.tensor.matmul(
                            out=pw[:], lhsT=m_sb[:cw, w * D:(w + 1) * D],
                            rhs=sd_sb[:cw, w * 128:(w + 1) * 128],
                            start=True, stop=(len(tiles_w) == 0))
                        for i, ti in enumerate(tiles_w):
                            gt, jj = gts[ti]
                            sg, js = sgs[ti]
                            nc.tensor.matmul(
                                out=pw[:], lhsT=gt[:, jj, :],
                                rhs=sg[:, js * D:(js + 1) * D],
                                start=False, stop=(i == len(tiles_w) - 1))
                        w0 = w * 128
                        nc.vector.tensor_copy(out=hagg[:, w0:w0 + cw],
                                              in_=pw[:, :cw])
                        nc.vector.reduce_sum(out=sums[:, w:w + 1],
                                             in_=hagg[:, w0:w0 + cw], axis=Axis.X)
                        sq = wk.tile([128, 128], f32, tag="sq")
                        nc.scalar.square(out=sq[:, :cw], in_=hagg[:, w0:w0 + cw])
                        nc.vector.reduce_sum(out=sumsq[:, w:w + 1],
                                             in_=sq[:, :cw], axis=Axis.X)

                # global BN stats (b_gcn cancels inside BN and is dropped)
                stat_sb = wk.tile([128, 2], f32, tag="stat")
                nc.vector.reduce_sum(out=stat_sb[:, 0:1], in_=sums[:], axis=Axis.X)
                nc.vector.reduce_sum(out=stat_sb[:, 1:2], in_=sumsq[:], axis=Axis.X)
                nc.scalar.dma_start(out=stat_in[:], in_=stat_sb[:])
                nc.gpsimd.collective_compute(
                    "AllReduce", Alu.add, replica_groups=rg,
                    ins=[stat_in.opt()], outs=[stat_out[l].opt()])
                stat_g = wk.tile([128, 2], f32, tag="statg")
                nc.scalar.dma_start(out=stat_g[:], in_=stat_out[l][:])

                mu = wk.tile([128, 1], f32, tag="mu")
                nc.vector.tensor_scalar(out=mu[:], in0=stat_g[:, 0:1],
                                        scalar1=1.0 / N, scalar2=None, op0=Alu.mult)
                ex2 = wk.tile([128, 1], f32, tag="ex2")
                nc.vector.tensor_scalar(out=ex2[:], in0=stat_g[:, 1:2],
                                        scalar1=1.0 / N, scalar2=None, op0=Alu.mult)
                musq = wk.tile([128, 1], f32, tag="musq")
                nc.vector.tensor_tensor(out=musq[:], in0=mu[:], in1=mu[:], op=Alu.mult)
                var = wk.tile([128, 1], f32, tag="var")
                nc.vector.tensor_tensor(out=var[:], in0=ex2[:], in1=musq[:],
                                        op=Alu.subtract)
                var2 = wk.tile([128, 1], f32, tag="var2")
                nc.vector.tensor_scalar(out=var2[:], in0=var[:], scalar1=float(EPS),
                                        scalar2=None, op0=Alu.add)
                stdv = wk.tile([128, 1], f32, tag="stdv")
                nc.scalar.activation(out=stdv[:], in_=var2[:], func=Act.Sqrt)
                rinv = wk.tile([128, 1], f32, tag="rinv")
                nc.vector.reciprocal(out=rinv[:], in_=stdv[:])
                a_t = wk.tile([128, 1], f32, tag="a_t")
                nc.vector.tensor_tensor(out=a_t[:], in0=gamma_sb[:, l:l + 1],
                                        in1=rinv[:], op=Alu.mult)
                t1 = wk.tile([128, 1], f32, tag="t1")
                nc.vector.tensor_tensor(out=t1[:], in0=mu[:], in1=a_t[:], op=Alu.mult)
                b2 = wk.tile([128, 1], f32, tag="b2")
                nc.vector.tensor_tensor(out=b2[:], in0=beta_sb[:, l:l + 1],
                                        in1=t1[:], op=Alu.subtract)

                # fused BN apply + residual (512-wide slabs to amortize
                # per-instruction overhead) + next-layer m (or final pool)
                for t0c in range(0, NT, 4):
                    tc_n = min(4, NT - t0c)
                    c0 = t0c * 128
                    cwc = sum(NT_W[t0c + i] for i in range(tc_n))
                    rl = wk.tile([128, 512], bf, tag="rl")
                    nc.scalar.activation(out=rl[:, :cwc], in_=hagg[:, c0:c0 + cwc],
                                         func=Act.Relu, bias=b2[:, 0:1],
                                         scale=a_t[:, 0:1])
                    nc.vector.tensor_add(out=h_out[:, c0:c0 + cwc],
                                         in0=rl[:, :cwc], in1=h_in[:, c0:c0 + cwc])
                    for t in range(t0c, t0c + tc_n):
                        bn_tail_tile(l, t, h_out)

            # ---- MLP readout ----
            pool_sb = wk.tile([G, D], f32, tag="pool_sb")
            nc.vector.tensor_copy(out=pool_sb[:], in_=ppool[:])
            nc.scalar.dma_start(out=pool_in[:], in_=pool_sb[:])
            nc.gpsimd.collective_compute(
                "AllReduce", Alu.add, replica_groups=rg,
                ins=[pool_in.opt()], outs=[pool_out.opt()])
            pg = wk.tile([G, D], f32, tag="pg")
            nc.scalar.dma_start(out=pg[:], in_=pool_out[:])
            hg = wk.tile([G, D], f32, tag="hg")
            nc.vector.tensor_scalar(out=hg[:], in0=pg[:], scalar1=invc_sb[:, 0:1],
                                    scalar2=None, op0=Alu.mult)
            pt2 = pmo.tile([128, G], f32, tag="pmlp")
            nc.tensor.matmul(out=pt2[:], lhsT=hg[:], rhs=ident_sb[:G, :G],
                             start=True, stop=True)
            hgT = wk.tile([128, G], f32, tag="hgT")
            nc.vector.tensor_copy(out=hgT[:], in_=pt2[:])
            pz1 = pmo.tile([D // 2, G], f32, tag="pmlp")
            nc.tensor.matmul(out=pz1[:], lhsT=wr1_sb[:], rhs=hgT[:],
                             start=True, stop=True)
            z1 = wk.tile([D // 2, G], f32, tag="z1")
            nc.scalar.activation(out=z1[:], in_=pz1[:], func=Act.Relu,
                                 bias=br1_sb[:, 0:1], scale=1.0)
            pz2 = pmo.tile([D // 4, G], f32, tag="pmlp")
            nc.tensor.matmul(out=pz2[:], lhsT=wr2_sb[:], rhs=z1[:],
                             start=True, stop=True)
            z2 = wk.tile([D // 4, G], f32, tag="z2")
            nc.scalar.activation(out=z2[:], in_=pz2[:], func=Act.Relu,
                                 bias=br2_sb[:, 0:1], scale=1.0)
            pz3 = pmo.tile([NC, G], f32, tag="pmlp")
            nc.tensor.matmul(out=pz3[:], lhsT=wr3_sb[:], rhs=z2[:],
                             start=True, stop=True)
            z3 = wk.tile([NC, G], f32, tag="z3")
            nc.scalar.activation(out=z3[:], in_=pz3[:], func=Act.Identity,
                                 bias=br3_sb[:, 0:1], scale=1.0)
            nc.sync.dma_start(out=out_p[:], in_=z3[:])

    nc.compile()
    return nc


_CACHE = {}


def kernel(x, edge_index, batch, W_emb, b_emb, W_gcn, b_gcn,
           bn_gamma, bn_beta, W_r1, b_r1, W_r2, b_r2, W_r3, b_r3):
    import os
    from concourse.bass_utils import run_bass_kernel_spmd
    trace = bool(os.environ.get("BASS_KERNEL_TRACE"))

    x = np.asarray(x, dtype=np.float32)
    meta, per_core, inv_cnt, bf16 = _preprocess(x, edge_index, batch)

    key = (meta["TILES"], meta["IDXCOLS"], tuple(map(tuple, meta["T"])))
    if key not in _CACHE:
        _CACHE[key] = _build(meta)
    nc = _CACHE[key]

    W_emb = np.asarray(W_emb, np.float32)
    shared = dict(
        wemb1=np.ascontiguousarray(W_emb[:128]).astype(bf16),
        wemb2=np.ascontiguousarray(W_emb[128:]).astype(bf16),
        b_emb=np.asarray(b_emb, np.float32).reshape(D, 1),
        W_gcn=np.asarray(W_gcn, np.float32).astype(bf16),
        gamma_t=np.ascontiguousarray(np.asarray(bn_gamma, np.float32).T),
        beta_t=np.ascontiguousarray(np.asarray(bn_beta, np.float32).T),
        W_r1=np.asarray(W_r1, np.float32),
        b_r1=np.asarray(b_r1, np.float32).reshape(D // 2, 1),
        W_r2=np.asarray(W_r2, np.float32),
        b_r2=np.asarray(b_r2, np.float32).reshape(D // 4, 1),
        W_r3=np.asarray(W_r3, np.float32),
        b_r3=np.asarray(b_r3, np.float32).reshape(NC, 1),
        ident=np.eye(128, dtype=np.float32),
        identb=np.eye(128, dtype=np.float32).astype(bf16),
        inv_cnt=inv_cnt,
    )
    in_maps = []
    for c in range(C):
        m = dict(shared)
        m.update(per_core[c])
        in_maps.append(m)

    res = run_bass_kernel_spmd(nc, in_maps, core_ids=list(range(C)), trace=trace)
    if trace and res.exec_time_ns is not None:
        print(f"HW exec time: {res.exec_time_ns} ns")
    out = res.results[0]["out"]  # [NC, G]
    return np.ascontiguousarray(out.T.astype(np.float32))  # [G, NC]


# revision 38
# speedup vs baseline: 1.0535x; 1.0033x over previous
"""GCNNet2 on 8 Trainium2 NeuronCores (Bass/Tile).

Strategy: shard nodes (contiguous 6250-node ranges) across 8 cores; each core
owns the aggregation for its dst range. The normalized adjacency is graph-
static, so the one-hot scatter matrices S (with the dst-side degree factor
folded in) are precomputed on host in bf16 and streamed from DRAM each layer.
m rows are pre-scaled by the src-side degree factor on-chip, so gathered rows
times S gives exactly norm-weighted messages; the b_gcn bias cancels inside
BatchNorm and is dropped. Per layer: m = h @ W (bf16) scaled by dinv, written
to two bounce halves that AllGather separately (half-A aggregation overlaps
the half-B collective); dma_gather calls of 8 edge tiles (1024 rows — the
SWDGE per-call index cap; larger calls hang) fetch
m[src] rows; PE accumulates seed (self-loop diag) + edge one-hot matmuls per
128-dst window in PSUM; BatchNorm via a tiny AllReduce of per-core sums;
fused scale/bias/relu + residual. Global mean pool via indicator matmul +
AllReduce, then the MLP readout (replicated).
"""
import numpy as np

# Problem constants (hardcoded per contract; kernel.py must be self-contained)
N = 50000
E = 800000
DIN = 146
D = 128
G = 64
L = 4
NC = 10
EPS = 1e-5

C = 8          # cores
NL = N // C    # 6250 nodes per core
NT = (NL + 127) // 128                # 49 node tiles (= aggregation windows)
NT_W = [min(128, NL - t * 128) for t in range(NT)]
HLOC = 3200    # local-node split: tiles 0-24 -> half A, 25-48 -> half B
TA = HLOC // 128          # 25 tiles in half A
HA = C * HLOC             # 25600 rows in table A (int16-safe)
HB = C * (NL - HLOC)      # 24400 rows in table B
GRP = 4        # windows per gather group
MAXJ = 8      # max edge tiles per dma_gather call (1024 idx cap; 1280+ hangs)


def _static_structure(counts):
    """counts: [C, NT, 2] per-core edge counts per (window, half).
    Returns core-invariant tile/call structure. Each call carries a
    core-invariant valid-index count V (max over cores): per-core idx
    streams hold [true edges][0-pads to V][-1 to 128j]; the SWDGE ucode
    emits exactly num_idxs_reg=V descriptors, skipping trailing -1 pads."""
    T = np.maximum.reduce(-(-counts // 128), axis=0)  # [NT, 2] ceil/max over cores
    groups = [list(range(g, min(g + GRP, NT))) for g in range(0, NT, GRP)]
    tile_base = {}
    tile_meta = []   # (w, h) per static tile
    gcalls = {}      # (gi, h) -> [(h, t0, j, V)]
    for gi, ws in enumerate(groups):
        for h in (0, 1):
            t0g = len(tile_meta)
            # max-over-core edge count for each bucket, laid out tile-contiguous
            gcnt = np.zeros(0, dtype=np.int64)
            for w in ws:
                tile_base[(w, h)] = len(tile_meta)
                tile_meta.extend([(w, h)] * int(T[w, h]))
                tc_ = np.zeros(int(T[w, h]), dtype=np.int64)
                mx = int(counts[:, w, h].max())
                full = mx // 128
                tc_[:full] = 128
                if full < len(tc_):
                    tc_[full] = mx - full * 128
                gcnt = np.concatenate([gcnt, tc_])
            ntg = len(tile_meta) - t0g
            cl = []
            r = 0
            while r < ntg:
                j = min(MAXJ, ntg - r)
                V = int(gcnt[r:r + j].sum())
                cl.append((h, t0g + r, j, max(V, 1)))
                r += j
            gcalls[(gi, h)] = cl
    TILES = len(tile_meta)
    # Software-pipelined issue order: PREF groups of half-A calls run before
    # the first half-B call, hiding the AG_B collective latency behind A-half
    # gather work. PREF is bounded by the gt/sg pool depth (buffer-reuse WAR
    # vs the in-order PE window chain): pos(B0) <= bufs + calls(g0,A).
    PREF = 1
    order = []
    for gi in range(min(PREF, len(groups))):
        order.extend(gcalls[(gi, 0)])
    for gi in range(len(groups)):
        order.extend(gcalls[(gi, 1)])
        if gi + PREF < len(groups):
            order.extend(gcalls[(gi + PREF, 0)])
    calls = []
    icol = 0
    for (h, t0, j, V) in order:
        calls.append((h, t0, j, icol, V))
        icol += 8 * j
    # map: static tile -> (call index, slot within call)
    tile_call = {}
    for ci, (h, t0, j, off, V) in enumerate(calls):
        for jj in range(j):
            tile_call[t0 + jj] = (ci, jj)
    return dict(T=T, groups=groups, tile_base=tile_base, tile_meta=tile_meta,
                calls=calls, TILES=TILES, IDXCOLS=icol, tile_call=tile_call)


def _preprocess(x, edge_index, batch):
    src = np.asarray(edge_index[0], dtype=np.int64)
    dst = np.asarray(edge_index[1], dtype=np.int64)
    batch = np.asarray(batch, dtype=np.int64)

    deg = (np.bincount(dst, minlength=N) + 1).astype(np.float32)  # + self-loop
    dinv = (1.0 / np.sqrt(deg)).astype(np.float32)

    # gather-table index (two tables split by owner-local offset)
    oc = src // NL
    osl = src % NL
    half = (osl >= HLOC).astype(np.int64)
    idx16 = np.where(half == 0, oc * HLOC + osl,
                     oc * (NL - HLOC) + (osl - HLOC)).astype(np.int16)

    core = dst // NL
    w = (dst % NL) // 128
    dstl = (dst % NL) % 128

    key = (core * NT + w) * 2 + half
    order = np.argsort(key, kind="stable")
    key_s = key[order]
    bounds = np.searchsorted(key_s, np.arange(C * NT * 2 + 1))
    counts = np.zeros((C, NT, 2), dtype=np.int64)
    for c in range(C):
        for ww in range(NT):
            for h in range(2):
                k = (c * NT + ww) * 2 + h
                counts[c, ww, h] = bounds[k + 1] - bounds[k]

    meta = _static_structure(counts)
    T, TILES, IDXCOLS = meta["T"], meta["TILES"], meta["IDXCOLS"]
    tile_base, calls = meta["tile_base"], meta["calls"]

    try:
        import ml_dtypes
        bf16 = ml_dtypes.bfloat16
    except ImportError:  # pragma: no cover
        from jax import numpy as jnp
        bf16 = jnp.bfloat16

    per_core = []
    for c in range(C):
        S = np.zeros((128, TILES * 128), dtype=np.float32)
        flat_idx = np.zeros((TILES, 128), dtype=np.int16)
        for ww in range(NT):
            for h in range(2):
                k = (c * NT + ww) * 2 + h
                el = order[bounds[k]:bounds[k + 1]]
                if len(el) == 0:
                    continue
                tb = tile_base[(ww, h)]
                s = np.arange(len(el))
                ti = tb + s // 128
                slot = s % 128
                S[slot, ti * 128 + dstl[el]] = dinv[dst[el]]
                flat_idx[ti, slot] = idx16[el]
        # pack gather indices per call: idx i of call -> [i%16, off + i//16]
        idx_arr = np.zeros((16, IDXCOLS), dtype=np.int16)
        for (h, t0, j, off, V) in calls:
            seq = flat_idx[t0:t0 + j].reshape(-1)
            idx_arr[:, off:off + 8 * j] = seq.reshape(8 * j, 16).T
        idx_rep = np.tile(idx_arr, (8, 1))

        lo = c * NL
        dinv_l = dinv[lo:lo + NL]
        dinv_p = np.zeros((128, NT), dtype=np.float32)
        sd = np.zeros((128, NT * 128), dtype=np.float32)
        Pm = np.zeros((128, NT * G), dtype=np.float32)
        for t in range(NT):
            cw = NT_W[t]
            dinv_p[:cw, t] = dinv_l[t * 128:t * 128 + cw]
            q = np.arange(cw)
            sd[q, t * 128 + q] = dinv_l[t * 128 + q]
            Pm[q, t * G + batch[lo + t * 128 + q]] = 1.0

        x_c = np.asarray(x[lo:lo + NL], dtype=np.float32).T  # [DIN, NL]
        per_core.append(dict(
            idx=idx_rep,
            S=np.ascontiguousarray(S).astype(bf16),
            sd=sd.astype(bf16),
            Pm=Pm.astype(bf16),
            dinv_p=dinv_p,
            x1_t=np.ascontiguousarray(x_c[:128]).astype(bf16),
            x2_t=np.ascontiguousarray(x_c[128:]).astype(bf16),
        ))

    cnt = np.bincount(batch, minlength=G).astype(np.float32)
    inv_cnt = (1.0 / np.maximum(cnt, 1.0)).astype(np.float32).reshape(G, 1)
    return meta, per_core, inv_cnt, bf16


def _build(meta):
    import concourse.bacc as bacc
    import concourse.mybir as mybir
    import concourse.tile as tile

    f32 = mybir.dt.float32
    bf = mybir.dt.bfloat16
    i16 = mybir.dt.int16
    Alu = mybir.AluOpType
    Act = mybir.ActivationFunctionType
    Axis = mybir.AxisListType

    TILES = meta["TILES"]
    IDXCOLS = meta["IDXCOLS"]
    tile_meta = meta["tile_meta"]
    calls = meta["calls"]
    groups = meta["groups"]
    tile_base = meta["tile_base"]
    T = meta["T"]
    tile_call = meta["tile_call"]

    nc = bacc.Bacc(None, target_bir_lowering=False, num_swdge_queues=4)

    P = {}
    P["x1_t"] = nc.declare_dram_parameter("x1_t", [128, NL], bf, isOutput=False)
    P["x2_t"] = nc.declare_dram_parameter("x2_t", [DIN - 128, NL], bf, isOutput=False)
    P["idx"] = nc.declare_dram_parameter("idx", [128, IDXCOLS], i16, isOutput=False)
    P["S"] = nc.declare_dram_parameter("S", [128, TILES * 128], bf, isOutput=False)
    P["sd"] = nc.declare_dram_parameter("sd", [128, NT * 128], bf, isOutput=False)
    P["Pm"] = nc.declare_dram_parameter("Pm", [128, NT * G], bf, isOutput=False)
    P["dinv_p"] = nc.declare_dram_parameter("dinv_p", [128, NT], f32, isOutput=False)
    P["wemb1"] = nc.declare_dram_parameter("wemb1", [128, D], bf, isOutput=False)
    P["wemb2"] = nc.declare_dram_parameter("wemb2", [DIN - 128, D], bf, isOutput=False)
    P["b_emb"] = nc.declare_dram_parameter("b_emb", [D, 1], f32, isOutput=False)
    P["W_gcn"] = nc.declare_dram_parameter("W_gcn", [L, D, D], bf, isOutput=False)
    P["gamma_t"] = nc.declare_dram_parameter("gamma_t", [D, L], f32, isOutput=False)
    P["beta_t"] = nc.declare_dram_parameter("beta_t", [D, L], f32, isOutput=False)
    P["W_r1"] = nc.declare_dram_parameter("W_r1", [D, D // 2], f32, isOutput=False)
    P["b_r1"] = nc.declare_dram_parameter("b_r1", [D // 2, 1], f32, isOutput=False)
    P["W_r2"] = nc.declare_dram_parameter("W_r2", [D // 2, D // 4], f32, isOutput=False)
    P["b_r2"] = nc.declare_dram_parameter("b_r2", [D // 4, 1], f32, isOutput=False)
    P["W_r3"] = nc.declare_dram_parameter("W_r3", [D // 4, NC], f32, isOutput=False)
    P["b_r3"] = nc.declare_dram_parameter("b_r3", [NC, 1], f32, isOutput=False)
    P["ident"] = nc.declare_dram_parameter("ident", [128, 128], f32, isOutput=False)
    P["identb"] = nc.declare_dram_parameter("identb", [128, 128], bf, isOutput=False)
    P["inv_cnt"] = nc.declare_dram_parameter("inv_cnt", [G, 1], f32, isOutput=False)
    out_p = nc.declare_dram_parameter("out", [NC, G], f32, isOutput=True)

    rg = [list(range(C))]

    with tile.TileContext(nc) as tc:
        with (
            tc.tile_pool(name="const", bufs=1) as cst,
            tc.tile_pool(name="hbuf", bufs=1) as hbuf,
            tc.tile_pool(name="gd", bufs=12) as gd,
            tc.tile_pool(name="sp", bufs=12) as sp,
            tc.tile_pool(name="work", bufs=3) as wk,
            tc.tile_pool(name="xst", bufs=1) as xst,
            tc.tile_pool(name="pag", bufs=4, space="PSUM") as pag,
            tc.tile_pool(name="pmm", bufs=2, space="PSUM") as pmm,
            tc.tile_pool(name="pmo", bufs=1, space="PSUM") as pmo,
            tc.tile_pool(name="dram", bufs=1, space="DRAM") as drp,
        ):
            def load_const(name, shape, dt=f32):
                t = cst.tile(shape, dt, tag=f"c_{name}")
                nc.sync.dma_start(out=t[:], in_=P[name][:])
                return t

            # embedding-critical loads first so h0/m/AG_A start ASAP;
            # aggregation/readout consts follow on the same queue.
            wemb1 = load_const("wemb1", [128, D], bf)
            wemb2 = load_const("wemb2", [DIN - 128, D], bf)
            bemb_sb = load_const("b_emb", [D, 1])
            dinv_sb = load_const("dinv_p", [128, NT])
            wgcn_sb = cst.tile([128, L * D], bf)
            for l in range(L):
                nc.sync.dma_start(out=wgcn_sb[:, l * D:(l + 1) * D],
                                  in_=P["W_gcn"][l])
            x1_sb = xst.tile([128, NL], bf, tag="x1")
            nc.sync.dma_start(out=x1_sb[:], in_=P["x1_t"][:])
            x2_sb = xst.tile([DIN - 128, NL], bf, tag="x2")
            nc.sync.dma_start(out=x2_sb[:], in_=P["x2_t"][:])
            idx_sb = load_const("idx", [128, IDXCOLS], i16)
            sd_sb = load_const("sd", [128, NT * 128], bf)
            pm_sb = load_const("Pm", [128, NT * G], bf)
            gamma_sb = load_const("gamma_t", [D, L])
            beta_sb = load_const("beta_t", [D, L])
            wr1_sb = load_const("W_r1", [D, D // 2])
            br1_sb = load_const("b_r1", [D // 2, 1])
            wr2_sb = load_const("W_r2", [D // 2, D // 4])
            br2_sb = load_const("b_r2", [D // 4, 1])
            wr3_sb = load_const("W_r3", [D // 4, NC])
            br3_sb = load_const("b_r3", [NC, 1])
            ident_sb = load_const("ident", [128, 128])
            identb_sb = load_const("identb", [128, 128], bf)
            invc_sb = load_const("inv_cnt", [G, 1])

            hA = hbuf.tile([128, NT * 128], bf)
            hB = hbuf.tile([128, NT * 128], bf)
            hagg = hbuf.tile([128, NT * 128], f32)
            m_sb = hbuf.tile([128, NT * 128], bf)
            sums = hbuf.tile([128, NT], f32)
            sumsq = hbuf.tile([128, NT], f32)

            m_bounceA = drp.tile([HLOC, D], bf)
            m_bounceB = drp.tile([NL - HLOC, D], bf)
            m_fullA = [drp.tile([HA, D], bf, name=f"m_fullA{l}",
                                addr_space="Shared") for l in range(L)]
            m_fullB = [drp.tile([HB, D], bf, name=f"m_fullB{l}",
                                addr_space="Shared") for l in range(L)]
            stat_in = drp.tile([128, 2], f32)
            stat_out = [drp.tile([128, 2], f32, name=f"stat_out{l}",
                                 addr_space="Shared") for l in range(L)]
            pool_in = drp.tile([G, D], f32)
            pool_out = drp.tile([G, D], f32, addr_space="Shared")

            # ---- embedding: h0_T = W_emb.T @ x_T + b_emb (x resident) ----
            hbufs = [hA, hB]
            qn = [0]

            def m_phase_tile(h_src, l, t):
                """m = dinv * (h @ W_l) for node tile t: PSUM -> bf16 m_sb ->
                bounce-half DMA; triggers the half AllGathers at t=TA-1/NT-1."""
                cw = NT_W[t]
                W_l = wgcn_sb[:, l * D:(l + 1) * D]
                pm = pmm.tile([128, D], f32, tag="pm", name="pm")
                nc.tensor.matmul(out=pm[:cw, :],
                                 lhsT=h_src[:, t * 128:t * 128 + cw],
                                 rhs=W_l, start=True, stop=True)
                nc.vector.tensor_scalar(out=m_sb[:cw, t * D:(t + 1) * D],
                                        in0=pm[:cw, :],
                                        scalar1=dinv_sb[:cw, t:t + 1],
                                        scalar2=None, op0=Alu.mult)
                dma_eng = nc.sync if t % 2 == 0 else nc.scalar
                if t < TA:
                    dma_eng.dma_start(
                        out=m_bounceA[t * 128:t * 128 + cw, :],
                        in_=m_sb[:cw, t * D:(t + 1) * D])
                else:
                    r0 = (t - TA) * 128
                    dma_eng.dma_start(
                        out=m_bounceB[r0:r0 + cw, :],
                        in_=m_sb[:cw, t * D:(t + 1) * D])
                if t == TA - 1:
                    nc.gpsimd.collective_compute(
                        "AllGather", Alu.bypass, replica_groups=rg,
                        ins=[m_bounceA.opt()], outs=[m_fullA[l].opt()])
                if t == NT - 1:
                    nc.gpsimd.collective_compute(
                        "AllGather", Alu.bypass, replica_groups=rg,
                        ins=[m_bounceB.opt()], outs=[m_fullB[l].opt()])

            # embedding + layer-0 m fused per tile
            for t in range(NT):
                c0 = t * 128
                cw = NT_W[t]
                pe = pmm.tile([128, 128], f32, tag="pm", name="pe")
                nc.tensor.matmul(out=pe[:, :cw], lhsT=wemb1[:],
                                 rhs=x1_sb[:, c0:c0 + cw], start=True, stop=False)
                nc.tensor.matmul(out=pe[:, :cw], lhsT=wemb2[:],
                                 rhs=x2_sb[:, c0:c0 + cw], start=False, stop=True)
                nc.scalar.activation(out=hA[:, c0:c0 + cw], in_=pe[:, :cw],
                                     func=Act.Identity, bias=bemb_sb[:, 0:1],
                                     scale=1.0)
                m_phase_tile(hA, 0, t)

            ppool = pmo.tile([G, D], f32, tag="ppool")

            def bn_tail_tile(l, t, h_out):
                """Per-tile tail after BN+residual: next-layer m, or (last
                layer) the pool transpose + indicator accumulation."""
                cw = NT_W[t]
                w0 = t * 128
                if l < L - 1:
                    m_phase_tile(h_out, l + 1, t)
                else:
                    pt = pmm.tile([128, 128], f32, tag="pm", name="pt")
                    nc.tensor.matmul(out=pt[:cw, :],
                                     lhsT=h_out[:, w0:w0 + cw],
                                     rhs=identb_sb[:], start=True, stop=True)
                    hr = wk.tile([128, 128], bf, tag="hr")
                    nc.scalar.activation(out=hr[:cw, :], in_=pt[:cw, :],
                                         func=Act.Identity, bias=0.0, scale=1.0)
                    nc.tensor.matmul(out=ppool[:],
                                     lhsT=pm_sb[:cw, t * G:(t + 1) * G],
                                     rhs=hr[:cw, :],
                                     start=(t == 0), stop=(t == NT - 1))

            # ---- GCN layers ----
            for l in range(L):
                h_in = hbufs[l % 2]
                h_out = hbufs[(l + 1) % 2]

                # issue all gathers + S streams (pipelined via pool bufs)
                gts = {}
                sgs = {}
                for ci, (h, t0, j, off, V) in enumerate(calls):
                    gt = gd.tile([128, MAXJ, D], bf, tag="gt")
                    tab = m_fullA[l] if h == 0 else m_fullB[l]
                    nc.gpsimd.dma_gather(
                        gt[:, :j, :], tab[:], idx_sb[:, off:off + 8 * j],
                        128 * j, 128 * j, D, queue_num=qn[0] % 4)
                    qn[0] += 1
                    sg = sp.tile([128, MAXJ * D], bf, tag="sg")
                    s_eng = nc.sync if ci % 2 == 0 else nc.scalar
                    s_eng.dma_start(out=sg[:, :j * D],
                                    in_=P["S"][:, t0 * D:(t0 + j) * D])
                    for jj in range(j):
                        gts[t0 + jj] = (gt, jj)
                        sgs[t0 + jj] = (sg, jj)

                # aggregate per window: seed opens PSUM, edge tiles accumulate
                for ws in groups:
                    for w in ws:
                        cw = NT_W[w]
                        tiles_w = (list(range(tile_base[(w, 0)],
                                              tile_base[(w, 0)] + int(T[w, 0])))
                                   + list(range(tile_base[(w, 1)],
                                                tile_base[(w, 1)] + int(T[w, 1]))))
                        pw = pag.tile([128, 128], f32, tag="pw")
                        nc.tensor.matmul(
                            out=pw[:], lhsT=m_sb[:cw, w * D:(w + 1) * D],
                            rhs=sd_sb[:cw, w * 128:(w + 1) * 128],
                            start=True, stop=(len(tiles_w) == 0))
                        for i, ti in enumerate(tiles_w):
                            gt, jj = gts[ti]
                            sg, js = sgs[ti]
                            nc.tensor.matmul(
                                out=pw[:], lhsT=gt[:, jj, :],
                                rhs=sg[:, js * D:(js + 1) * D],
                                start=False, stop=(i == len(tiles_w) - 1))
                        w0 = w * 128
                        nc.vector.tensor_copy(out=hagg[:, w0:w0 + cw],
                                              in_=pw[:, :cw])
                        nc.vector.reduce_sum(out=sums[:, w:w + 1],
                                             in_=hagg[:, w0:w0 + cw], axis=Axis.X)
                        sq = wk.tile([128, 128], f32, tag="sq")
                        nc.scalar.square(out=sq[:, :cw], in_=hagg[:, w0:w0 + cw])
                        nc.vector.reduce_sum(out=sumsq[:, w:w + 1],
                                             in_=sq[:, :cw], axis=Axis.X)

                # global BN stats (b_gcn cancels inside BN and is dropped)
                stat_sb = wk.tile([128, 2], f32, tag="stat")
                nc.vector.reduce_sum(out=stat_sb[:, 0:1], in_=sums[:], axis=Axis.X)
                nc.vector.reduce_sum(out=stat_sb[:, 1:2], in_=sumsq[:], axis=Axis.X)
                nc.scalar.dma_start(out=stat_in[:], in_=stat_sb[:])
                nc.gpsimd.collective_compute(
                    "AllReduce", Alu.add, replica_groups=rg,
                    ins=[stat_in.opt()], outs=[stat_out[l].opt()])
                stat_g = wk.tile([128, 2], f32, tag="statg")
                nc.scalar.dma_start(out=stat_g[:], in_=stat_out[l][:])

                mu = wk.tile([128, 1], f32, tag="mu")
                nc.vector.tensor_scalar(out=mu[:], in0=stat_g[:, 0:1],
                                        scalar1=1.0 / N, scalar2=None, op0=Alu.mult)
                ex2 = wk.tile([128, 1], f32, tag="ex2")
                nc.vector.tensor_scalar(out=ex2[:], in0=stat_g[:, 1:2],
                                        scalar1=1.0 / N, scalar2=None, op0=Alu.mult)
                musq = wk.tile([128, 1], f32, tag="musq")
                nc.vector.tensor_tensor(out=musq[:], in0=mu[:], in1=mu[:], op=Alu.mult)
                var = wk.tile([128, 1], f32, tag="var")
                nc.vector.tensor_tensor(out=var[:], in0=ex2[:], in1=musq[:],
                                        op=Alu.subtract)
                var2 = wk.tile([128, 1], f32, tag="var2")
                nc.vector.tensor_scalar(out=var2[:], in0=var[:], scalar1=float(EPS),
                                        scalar2=None, op0=Alu.add)
                stdv = wk.tile([128, 1], f32, tag="stdv")
                nc.scalar.activation(out=stdv[:], in_=var2[:], func=Act.Sqrt)
                rinv = wk.tile([128, 1], f32, tag="rinv")
                nc.vector.reciprocal(out=rinv[:], in_=stdv[:])
                a_t = wk.tile([128, 1], f32, tag="a_t")
                nc.vector.tensor_tensor(out=a_t[:], in0=gamma_sb[:, l:l + 1],
                                        in1=rinv[:], op=Alu.mult)
                t1 = wk.tile([128, 1], f32, tag="t1")
                nc.vector.tensor_tensor(out=t1[:], in0=mu[:], in1=a_t[:], op=Alu.mult)
                b2 = wk.tile([128, 1], f32, tag="b2")
                nc.vector.tensor_tensor(out=b2[:], in0=beta_sb[:, l:l + 1],
                                        in1=t1[:], op=Alu.subtract)

                # fused BN apply + residual (512-wide slabs to amortize
                # per-instruction overhead) + next-layer m (or final pool)
                for t0c in range(0, NT, 4):
                    tc_n = min(4, NT - t0c)
                    c0 = t0c * 128
                    cwc = sum(NT_W[t0c + i] for i in range(tc_n))
                    rl = wk.tile([128, 512], bf, tag="rl")
                    nc.scalar.activation(out=rl[:, :cwc], in_=hagg[:, c0:c0 + cwc],
                                         func=Act.Relu, bias=b2[:, 0:1],
                                         scale=a_t[:, 0:1])
                    nc.vector.tensor_add(out=h_out[:, c0:c0 + cwc],
                                         in0=rl[:, :cwc], in1=h_in[:, c0:c0 + cwc])
                    for t in range(t0c, t0c + tc_n):
                        bn_tail_tile(l, t, h_out)

            # ---- MLP readout ----
            pool_sb = wk.tile([G, D], f32, tag="pool_sb")
            nc.vector.tensor_copy(out=pool_sb[:], in_=ppool[:])
            nc.scalar.dma_start(out=pool_in[:], in_=pool_sb[:])
            nc.gpsimd.collective_compute(
                "AllReduce", Alu.add, replica_groups=rg,
                ins=[pool_in.opt()], outs=[pool_out.opt()])
            pg = wk.tile([G, D], f32, tag="pg")
            nc.scalar.dma_start(out=pg[:], in_=pool_out[:])
            hg = wk.tile([G, D], f32, tag="hg")
            nc.vector.tensor_scalar(out=hg[:], in0=pg[:], scalar1=invc_sb[:, 0:1],
                                    scalar2=None, op0=Alu.mult)
            pt2 = pmo.tile([128, G], f32, tag="pmlp")
            nc.tensor.matmul(out=pt2[:], lhsT=hg[:], rhs=ident_sb[:G, :G],
                             start=True, stop=True)
            hgT = wk.tile([128, G], f32, tag="hgT")
            nc.vector.tensor_copy(out=hgT[:], in_=pt2[:])
            pz1 = pmo.tile([D // 2, G], f32, tag="pmlp")
            nc.tensor.matmul(out=pz1[:], lhsT=wr1_sb[:], rhs=hgT[:],
                             start=True, stop=True)
            z1 = wk.tile([D // 2, G], f32, tag="z1")
            nc.scalar.activation(out=z1[:], in_=pz1[:], func=Act.Relu,
                                 bias=br1_sb[:, 0:1], scale=1.0)
            pz2 = pmo.tile([D // 4, G], f32, tag="pmlp")
            nc.tensor.matmul(out=pz2[:], lhsT=wr2_sb[:], rhs=z1[:],
                             start=True, stop=True)
            z2 = wk.tile([D // 4, G], f32, tag="z2")
            nc.scalar.activation(out=z2[:], in_=pz2[:], func=Act.Relu,
                                 bias=br2_sb[:, 0:1], scale=1.0)
            pz3 = pmo.tile([NC, G], f32, tag="pmlp")
            nc.tensor.matmul(out=pz3[:], lhsT=wr3_sb[:], rhs=z2[:],
                             start=True, stop=True)
            z3 = wk.tile([NC, G], f32, tag="z3")
            nc.scalar.activation(out=z3[:], in_=pz3[:], func=Act.Identity,
                                 bias=br3_sb[:, 0:1], scale=1.0)
            nc.sync.dma_start(out=out_p[:], in_=z3[:])

    nc.compile()
    return nc


_CACHE = {}


def kernel(x, edge_index, batch, W_emb, b_emb, W_gcn, b_gcn,
           bn_gamma, bn_beta, W_r1, b_r1, W_r2, b_r2, W_r3, b_r3):
    import os
    from concourse.bass_utils import run_bass_kernel_spmd
    trace = bool(os.environ.get("BASS_KERNEL_TRACE"))

    x = np.asarray(x, dtype=np.float32)
    meta, per_core, inv_cnt, bf16 = _preprocess(x, edge_index, batch)

    key = (meta["TILES"], meta["IDXCOLS"], tuple(map(tuple, meta["T"])))
    if key not in _CACHE:
        _CACHE[key] = _build(meta)
    nc = _CACHE[key]

    W_emb = np.asarray(W_emb, np.float32)
    shared = dict(
        wemb1=np.ascontiguousarray(W_emb[:128]).astype(bf16),
        wemb2=np.ascontiguousarray(W_emb[128:]).astype(bf16),
        b_emb=np.asarray(b_emb, np.float32).reshape(D, 1),
        W_gcn=np.asarray(W_gcn, np.float32).astype(bf16),
        gamma_t=np.ascontiguousarray(np.asarray(bn_gamma, np.float32).T),
        beta_t=np.ascontiguousarray(np.asarray(bn_beta, np.float32).T),
        W_r1=np.asarray(W_r1, np.float32),
        b_r1=np.asarray(b_r1, np.float32).reshape(D // 2, 1),
        W_r2=np.asarray(W_r2, np.float32),
        b_r2=np.asarray(b_r2, np.float32).reshape(D // 4, 1),
        W_r3=np.asarray(W_r3, np.float32),
        b_r3=np.asarray(b_r3, np.float32).reshape(NC, 1),
        ident=np.eye(128, dtype=np.float32),
        identb=np.eye(128, dtype=np.float32).astype(bf16),
        inv_cnt=inv_cnt,
    )
    in_maps = []
    for c in range(C):
        m = dict(shared)
        m.update(per_core[c])
        in_maps.append(m)

    res = run_bass_kernel_spmd(nc, in_maps, core_ids=list(range(C)), trace=trace)
    if trace and res.exec_time_ns is not None:
        print(f"HW exec time: {res.exec_time_ns} ns")
    out = res.results[0]["out"]  # [NC, G]
    return np.ascontiguousarray(out.T.astype(np.float32))  # [G, NC]
